# revision 24
# baseline (speedup 1.0000x reference)
"""Multi-head attention (S=2048, D=1024, H=16, dk=dv=64) on 8 TRN2 NeuronCores.

Head-parallel tensor parallelism: core c owns heads {2c, 2c+1} (FW=128
features). All encodings + weights are cast to bf16 on the HOST, halving
input DMA (12MB/core) and removing all on-device casts.

Layouts (per core):
  kt_sb/qt_sb [128=(h,dk), S]  -- K/Q projections, feature-major.
  v_sb [128=t, 16, 128=(h,dv)] -- V t-major (PE-transposed per quarter).
  scores: per (chunk ci, t-tile tt): two K=64 matmuls ROW-TILED at
    tile_position (0,0)/(64,0) run concurrently -> mega psum [128,1024].
  exp on ACT (scale 1/8) -> ex bf16 [128, (h,s)=1024].
  ctx: per (ci, k): two M=64 matmuls COL-TILED at (0,0)/(0,64) run
    concurrently, accumulating [128=(h,dv), 512] over k=0..15.
  softmax denominators: DVE accumulates den_acc[ci] += ex tile (bf16),
    then a [128,1]-ones matmul partition-reduces each head's half.
  normalize -> cat_loc bf16 -> per-chunk AllGather -> out-proj column
    slice per core -> outT [128, S] f32. Host concat + transpose.

Slot pipeline: 64 slots (4 chunks x 16 t-tiles), ACT-paced. V-proj and
Q quarters 2/3 project inside chunk 0/1 slots; ctx for chunk ci starts
at slot CTX_START[ci] (lag covers V readiness and exp latency); per-chunk
AllGathers fire as soon as each chunk's ctx drains so only the last AG
sits on the tail.
"""

import numpy as np
import ml_dtypes

import concourse.bass as bass
import concourse.mybir as mybir
import concourse.tile as tile
from concourse import bacc
from concourse.bass_utils import run_bass_kernel_spmd
from concourse.masks import make_identity

S = 2048
D = 1024
H = 16
DK = 64
DV = 64
NCORES = 8
HPC = H // NCORES          # heads per core = 2
FW = HPC * DV              # per-core feature width = 128
P = 128                    # partitions
KT_D = D // P              # 8 contraction tiles over D
TT = S // P                # 16 tiles over t (keys)
NQ = 512                   # projection moving free dim
CW = 512                   # s-chunk width (scores/ctx/AG granularity)
NCH = S // CW              # 4 chunks

F32 = mybir.dt.float32
BF16 = mybir.dt.bfloat16
EXPF = mybir.ActivationFunctionType.Exp

# ctx(ci, k) is emitted in slot CTX_START[ci] + k
CTX_START = (22, 30, 42, 49)

_cache = {}


def _prep_w(w):
    """[D, FW] -> [128, KT_D*FW]: row p holds all d-tiles' row p."""
    return np.ascontiguousarray(
        np.transpose(w.reshape(KT_D, P, FW), (1, 0, 2)).reshape(P, KT_D * FW)
    )


def build():
    nc = bacc.Bacc(None, target_bir_lowering=False)

    enc_in = {
        x: nc.dram_tensor(f"enc{x}_t", [D, S], BF16, kind="ExternalInput")
        for x in ("q", "k", "v")
    }
    w_in = {
        n: nc.dram_tensor(n, [P, KT_D * FW], BF16, kind="ExternalInput")
        for n in ("wq", "wk", "wv", "wo")
    }
    out_t = nc.dram_tensor("outT", [FW, S], F32, kind="ExternalOutput")

    with tile.TileContext(nc) as tc:
        with (
            tc.tile_pool(name="wts", bufs=1) as wts,
            tc.tile_pool(name="encp", bufs=1) as encp,
            tc.tile_pool(name="qkv", bufs=1) as qkv,
            tc.tile_pool(name="expp", bufs=1) as expp,
            tc.tile_pool(name="misc", bufs=1) as misc,
            tc.tile_pool(name="dram", bufs=1, space="DRAM") as dram,
        ):
            rg = [list(range(NCORES))]
            # q0 = HWDGE (sync engine): K/Q streams + V first half +
            # collective staging. qs = SWDGE (gpsimd): weights wv/wo +
            # V second half. Scalar issues NO DMA so ACT does exp only.
            q0, qs = nc.sync, nc.gpsimd
            from concourse.bass import _add_dep_helper

            # HAM warm-up on garbage data: no DMA dependency, so the PE
            # clock reaches 8/8 before the first real projection.
            dummy = misc.tile([P, NQ], BF16, tag="dummy", name="dummy")
            nc.vector.memset(dummy[:], 0.0)

            # ---------------- DMA emission (priority order) ----------
            # Batched >=1MB transfers: one InstDMACopy spreads across all
            # 16 SDMA engines; small per-tile DMAs are fixed-cost bound.
            wt = {}
            for name, q in (("wk", q0), ("wq", q0), ("wv", qs), ("wo", qs)):
                wt[name] = wts.tile([P, KT_D, FW], BF16, tag=f"w{name}",
                                    name=name)
                q.dma_start(wt[name].rearrange("p k m -> p (k m)"),
                            w_in[name][:])

            ek_src = enc_in["k"][:].rearrange("(dt p) s -> p dt s", p=P)
            ek_all = encp.tile([P, KT_D, S], BF16, tag="ek", name="ek_all")
            for i in range(4):
                q0.dma_start(ek_all[:, 2 * i: 2 * i + 2, :],
                             ek_src[:, 2 * i: 2 * i + 2, :])

            eq_src = enc_in["q"][:].rearrange("(dt p) s -> p dt s", p=P)
            eq_all = encp.tile([P, KT_D, S], BF16, tag="eq", name="eq_all")
            q0.dma_start(eq_all[:, :, 0:NQ], eq_src[:, :, 0:NQ])
            d_q1 = q0.dma_start(eq_all[:, :, NQ: 2 * NQ],
                                eq_src[:, :, NQ: 2 * NQ])

            ev_src = enc_in["v"][:].rearrange("(dt p) s -> p dt s", p=P)
            ev_all = encp.tile([P, KT_D, S], BF16, tag="ev", name="ev_all")
            q0.dma_start(ev_all[:, 0:4, :], ev_src[:, 0:4, :])
            d_vb = qs.dma_start(ev_all[:, 4:8, :], ev_src[:, 4:8, :])
            _add_dep_helper(d_vb.ins, d_q1.ins, sync=True,
                            reason="defer V 2nd half behind K/Q0/Q1")

            q0.dma_start(eq_all[:, :, 2 * NQ: 3 * NQ],
                         eq_src[:, :, 2 * NQ: 3 * NQ])
            q0.dma_start(eq_all[:, :, 3 * NQ: 4 * NQ],
                         eq_src[:, :, 3 * NQ: 4 * NQ])

            # ---------------- persistent SBUF state ------------------
            kt_sb = qkv.tile([P, S], BF16, tag="kt")
            qt_sb = qkv.tile([P, S], BF16, tag="qt")
            v_sb = qkv.tile([P, TT, FW], BF16, tag="vsb")
            cat_loc = qkv.tile([P, S], BF16, tag="cat")
            ident = wts.tile([P, P], BF16, tag="ident")
            make_identity(nc, ident)
            ones_col = wts.tile([P, 1], BF16, tag="ones")
            nc.any.memset(ones_col[:], 1.0)

            # ---------------- phase 0: warm-up, K proj, Q quarter 0 --
            ps_p_cm = tc.tile_pool(name="ps_p", bufs=1, space="PSUM")
            ps_p = ps_p_cm.__enter__()
            wm = ps_p.tile([P, NQ], F32, tag="warm", name="wm")
            for _ in range(10):
                nc.tensor.matmul(wm[:], dummy[:, 0:P], dummy[:],
                                 start=True, stop=True)
            kacc = {
                sc: ps_p.tile([P, NQ], F32, tag=f"ka{sc}", name=f"ka{sc}")
                for sc in range(4)
            }
            for dt in range(KT_D):
                for sc in range(4):
                    nc.tensor.matmul(
                        kacc[sc][:], wt["wk"][:, dt, :],
                        ek_all[:, dt, sc * NQ: (sc + 1) * NQ],
                        start=(dt == 0), stop=(dt == KT_D - 1))
            for sc in range(4):
                nc.scalar.copy(kt_sb[:, sc * NQ: (sc + 1) * NQ], kacc[sc][:])
            qq0ps = ps_p.tile([P, NQ], F32, tag="qq0", name="qq0")
            for dt in range(KT_D):
                nc.tensor.matmul(qq0ps[:], wt["wq"][:, dt, :],
                                 eq_all[:, dt, 0:NQ],
                                 start=(dt == 0), stop=(dt == KT_D - 1))
            nc.scalar.copy(qt_sb[:, 0:NQ], qq0ps[:])
            ps_p_cm.__exit__(None, None, None)

            # ---------------- phase 1 pools ---------------------------
            ps_mega_cm = tc.tile_pool(name="ps_mega", bufs=2, space="PSUM")
            ps_mega = ps_mega_cm.__enter__()
            ps_ctx_cm = tc.tile_pool(name="ps_ctx", bufs=1, space="PSUM")
            ps_ctx = ps_ctx_cm.__enter__()
            ps_sh_cm = tc.tile_pool(name="ps_sh", bufs=1, space="PSUM")
            ps_sh = ps_sh_cm.__enter__()
            ps_fx_cm = tc.tile_pool(name="ps_fx", bufs=1, space="PSUM")
            ps_fx = ps_fx_cm.__enter__()

            exs = {}
            ctx_ps = {}
            den_acc = {}
            gas = {}

            def scores_exp(ci, tt):
                m = ps_mega.tile([P, 2 * NQ], F32, tag="mega", name="m")
                s0 = ci * CW
                for h in range(HPC):
                    nc.tensor.matmul(
                        m[:, h * NQ: (h + 1) * NQ],
                        kt_sb[h * DK: (h + 1) * DK, tt * P: (tt + 1) * P],
                        qt_sb[h * DK: (h + 1) * DK, s0: s0 + NQ],
                        start=True, stop=True,
                        tile_position=(h * DK, 0))
                ex = expp.tile([P, 2 * NQ], BF16, tag="ex", bufs=24,
                               name="ex")
                nc.scalar.activation(ex[:], m[:], EXPF, scale=1.0 / 8.0)
                exs[(ci, tt)] = ex

            def ctx_op(cj, k):
                if k == 0:
                    ctx_ps[cj] = ps_ctx.tile([P, CW], F32, tag="ctx",
                                             bufs=2, name=f"ctx{cj}")
                ex = exs[(cj, k)]
                for h in range(HPC):
                    nc.tensor.matmul(
                        ctx_ps[cj][h * DV: (h + 1) * DV, :],
                        v_sb[:, k, h * DV: (h + 1) * DV],
                        ex[:, h * NQ: (h + 1) * NQ],
                        start=(k == 0), stop=(k == TT - 1),
                        tile_position=(0, h * DV))
                # denominator accumulation on DVE (bf16)
                if k == 0:
                    den_acc[cj] = qkv.tile([P, 2 * NQ], BF16, tag="den",
                                           bufs=2, name=f"den{cj}")
                    nc.vector.tensor_copy(den_acc[cj][:], ex[:])
                else:
                    nc.vector.tensor_add(den_acc[cj][:], den_acc[cj][:],
                                         ex[:])

            def norm_and_ag(cj):
                c0 = cj * CW
                for h in range(HPC):
                    dps = ps_sh.tile([1, CW], F32, tag="sh1", bufs=1,
                                     name=f"dps{cj}{h}")
                    nc.tensor.matmul(
                        dps[:], ones_col[:],
                        den_acc[cj][:, h * NQ: (h + 1) * NQ],
                        start=True, stop=True)
                    den_sb = misc.tile([1, CW], F32, tag="densb", bufs=1,
                                       name="den_sb")
                    nc.vector.tensor_copy(den_sb[:], dps[:])
                    recip = misc.tile([1, CW], F32, tag="recip", bufs=1,
                                      name="recip")
                    nc.vector.reciprocal_approx_fast(recip[:], den_sb[:])
                    bcast = misc.tile([DV, CW], F32, tag="bcast", bufs=2,
                                      name="bcast")
                    nc.gpsimd.partition_broadcast(bcast[:], recip[:])
                    nc.vector.tensor_mul(
                        cat_loc[h * DV: (h + 1) * DV, c0: c0 + CW],
                        ctx_ps[cj][h * DV: (h + 1) * DV, :],
                        bcast[:])
                cb = dram.tile([P, CW], BF16, tag=f"catb{cj}", name="cb")
                qs.dma_start(cb[:], cat_loc[:, c0: c0 + CW])
                ga = dram.tile([D, CW], BF16, tag=f"catall{cj}", name="ga")
                nc.gpsimd.collective_compute(
                    "AllGather", mybir.AluOpType.bypass,
                    ins=[cb[:].opt()], outs=[ga[:].opt()],
                    replica_groups=rg)
                gas[cj] = ga

            catins = {}

            def catin_load(cj):
                ct = encp.tile([P, KT_D, CW], BF16, tag="catin", bufs=2,
                               name="ct")
                q0.dma_start(
                    ct[:], gas[cj][:].rearrange("(kt p) s -> p kt s", p=P))
                catins[cj] = ct

            def outproj_mms(cj):
                c0 = cj * CW
                m = ps_fx.tile([P, CW], F32, tag="fx", bufs=1, name="om")
                for kt in range(KT_D):
                    nc.tensor.matmul(m[:], wt["wo"][:, kt, :],
                                     catins[cj][:, kt, :],
                                     start=(kt == 0), stop=(kt == KT_D - 1))
                ob = misc.tile([P, CW], F32, tag="ob", bufs=1, name="ob")
                nc.vector.tensor_copy(ob[:], m[:])
                q0.dma_start(out_t[:, c0: c0 + CW], ob[:])

            vaccs = {}

            def vproj_half(qv, half):
                if half == 0:
                    vaccs[qv] = ps_fx.tile([P, NQ], F32, tag="fx", bufs=1,
                                           name=f"vacc{qv}")
                for dt in range(4 * half, 4 * half + 4):
                    nc.tensor.matmul(
                        vaccs[qv][:], wt["wv"][:, dt, :],
                        ev_all[:, dt, qv * NQ: (qv + 1) * NQ],
                        start=(dt == 0), stop=(dt == KT_D - 1))
                if half == 1:
                    vt_q = misc.tile([P, NQ], BF16, tag="vtq", bufs=2,
                                     name=f"vtq{qv}")
                    nc.vector.tensor_copy(vt_q[:], vaccs[qv][:])
                    return vt_q
                return None

            def vproj_transposes(qv, vt_q):
                for j in range(4):
                    tp = ps_fx.tile([P, P], BF16, tag="fx", bufs=1,
                                    name=f"tp{qv}{j}")
                    nc.tensor.transpose(tp[:], vt_q[:, j * P: (j + 1) * P],
                                        ident[:])
                    nc.vector.tensor_copy(v_sb[:, 4 * qv + j, :], tp[:])

            # qq2/qq3 projection state
            qq_t = {}

            def qq_mm(qq, dt):
                if dt == 0:
                    qq_t[qq] = ps_sh.tile([P, NQ], F32, tag="sh1", bufs=1,
                                          name=f"qq{qq}")
                nc.tensor.matmul(qq_t[qq][:], wt["wq"][:, dt, :],
                                 eq_all[:, dt, qq * NQ: (qq + 1) * NQ],
                                 start=(dt == 0), stop=(dt == KT_D - 1))
                if dt == KT_D - 1:
                    nc.vector.tensor_copy(
                        qt_sb[:, qq * NQ: (qq + 1) * NQ], qq_t[qq][:])

            # ---------------- the 64-slot pipeline --------------------
            vt_qs = {}
            for g in range(64):
                ci, tt = g // 16, g % 16
                scores_exp(ci, tt)
                # Q quarter 1: one d-tile matmul per slot 2..9
                if 2 <= g < 10:
                    qq_mm(1, g - 2)
                # V projection: each quarter spread over 3 slots
                # (vacc dt0-3 | vacc dt4-7 + copy | 4 transposes)
                if 10 <= g < 22:
                    qv, ph = (g - 10) // 3, (g - 10) % 3
                    if ph == 0:
                        vproj_half(qv, 0)
                    elif ph == 1:
                        vt_qs[qv] = vproj_half(qv, 1)
                    else:
                        vproj_transposes(qv, vt_qs[qv])
                # Q quarters 2,3: one d-tile matmul per slot
                if 22 <= g < 30:
                    qq_mm(2, g - 22)
                elif 30 <= g < 38:
                    qq_mm(3, g - 30)
                # ctx + denominator pipeline
                for cj in range(NCH):
                    k = g - CTX_START[cj]
                    if 0 <= k < TT:
                        ctx_op(cj, k)
                # normalize + AllGather as soon as a chunk's ctx drains
                for cj in range(NCH):
                    if g == CTX_START[cj] + TT:
                        norm_and_ag(cj)

            # ---------------- tail: out-projections -------------------
            # All gather-dependent work lives here so a late AllGather
            # can never stall the PE FIFO mid-pipeline.
            ctx_op(3, 15)
            norm_and_ag(3)
            catin_load(0)
            catin_load(1)
            outproj_mms(0)
            catin_load(2)
            outproj_mms(1)
            catin_load(3)
            outproj_mms(2)
            outproj_mms(3)

            ps_fx_cm.__exit__(None, None, None)
            ps_sh_cm.__exit__(None, None, None)
            ps_ctx_cm.__exit__(None, None, None)
            ps_mega_cm.__exit__(None, None, None)

    nc.compile()
    return nc


def kernel(
    encodings_for_q,
    encodings_for_k,
    encodings_for_v,
    W_q,
    W_k,
    W_v,
    W_out,
    _trace: bool = False,
):
    BF = ml_dtypes.bfloat16
    eqT = np.ascontiguousarray(
        np.asarray(encodings_for_q, dtype=np.float32).T).astype(BF)
    ekT = np.ascontiguousarray(
        np.asarray(encodings_for_k, dtype=np.float32).T).astype(BF)
    evT = np.ascontiguousarray(
        np.asarray(encodings_for_v, dtype=np.float32).T).astype(BF)
    W_q = np.asarray(W_q, dtype=np.float32)
    W_k = np.asarray(W_k, dtype=np.float32)
    W_v = np.asarray(W_v, dtype=np.float32)
    W_out = np.asarray(W_out, dtype=np.float32)

    if "nc" not in _cache:
        _cache["nc"] = build()
    nc = _cache["nc"]

    in_maps = []
    for c in range(NCORES):
        hs = slice(HPC * c, HPC * (c + 1))
        in_maps.append(
            {
                "encq_t": eqT,
                "enck_t": ekT,
                "encv_t": evT,
                "wq": _prep_w(
                    np.transpose(W_q[hs], (1, 0, 2)).reshape(D, FW)
                ).astype(BF),
                "wk": _prep_w(
                    np.transpose(W_k[hs], (1, 0, 2)).reshape(D, FW)
                ).astype(BF),
                "wv": _prep_w(
                    np.transpose(W_v[hs], (1, 0, 2)).reshape(D, FW)
                ).astype(BF),
                "wo": _prep_w(W_out[:, FW * c: FW * (c + 1)]).astype(BF),
            }
        )

    r = run_bass_kernel_spmd(
        nc, in_maps, core_ids=list(range(NCORES)), trace=_trace
    )
    out = np.concatenate(
        [r.results[c]["outT"].T for c in range(NCORES)], axis=1
    )
    if _trace:
        kernel.last_exec_time_ns = r.exec_time_ns
        kernel.last_insts = (
            r.instructions_and_trace[0] if r.instructions_and_trace else None
        )
    return out.astype(np.float32)
